# revision 18
# baseline (speedup 1.0000x reference)
"""GatedBlock kernel for Trainium2, data-parallel across 8 NeuronCores.

Math (per row of x[N, 960], irreps 256x0e | 128x1o | 64x2e):
  y0 = x0 @ W0 / sqrt(256)          -> [384] = 256 scalars + 128 gates
  y1 = einsum('mc,mk->kc', x1, W1) / sqrt(128)   -> [64, 3]
  y2 = einsum('mc,mk->kc', x2, W2) / sqrt(64)    -> [64, 5]
  out = [silu(scalars)*1.6791, y1*sig(g1)*1.8484, y2*sig(g2)*1.8484]  -> [768]

Device strategy (per core, 25088 rows after padding):
  - Host pre-transposes x to channel-major fp16 "xT" [960, R] with the
    (2l+1) component axis grouped c-major so each einsum component is a
    dense K<=128 matmul.  Path norms are folded into the fp16 weights;
    SIGMOID_NORM is folded into W1/W2 (y*s*sig == (y*s)*sig).
  - Rows ride the PSUM partition axis: out[128 rows, cols] = xT_chunk.T @ W,
    so the output is written row-major and needs no host transpose.
  - y2 components are packed in pairs via a block-diagonal W2 so K=128.
  - ScalarE applies Silu / Sigmoid straight out of PSUM; VectorE applies
    the gates with strided APs that interleave (k, c) into the reference
    column order during the multiply.
"""
import os
import numpy as np

N = 200000
NCORES = 8
R = 25088          # rows per core (200704 = 8*25088, N padded with zeros)
GROUP = 3584       # rows per x-load group (must divide R, multiple of 512)
SILU_NORM = 1.6791
SIGMOID_NORM = 1.8484
INV0 = 1.0 / np.sqrt(256.0)
INV1 = 1.0 / np.sqrt(128.0)
INV2 = 1.0 / np.sqrt(64.0)

# channel permutation: component axis grouped c-major per irrep block
_PERM = np.concatenate([
    np.arange(256),
    np.array([256 + 3 * m + c for c in range(3) for m in range(128)]),
    np.array([640 + 5 * m + c for c in range(5) for m in range(64)]),
]).astype(np.int32)

# device stores y1/y2 c-major; map final column -> device column
_COL_INV = np.concatenate([
    np.arange(256),
    np.array([256 + c * 64 + k for k in range(64) for c in range(3)]),
    np.array([448 + c * 64 + k for k in range(64) for c in range(5)]),
]).astype(np.int32)

# Scalars use the Silu LUT; gates use sigmoid(g) = 0.5 + 0.5*tanh(g/2)
# because silu+tanh share one ACT table set ("silu_and_others") while
# silu+sigmoid do not — alternating table sets costs an ACT_TABLE_LOAD
# (~1.3us) per pair, 126us/core.  CoreSim lacks the Silu LUT, so the sim
# build (silu_on_act=False) uses z*sigmoid(z/c) with c folded into W0.
SILU_ON_ACT = True

LAST_RESULT = None  # BassKernelResults of the most recent run (for test.py)


def build_nc(rows, group, silu_on_act=SILU_ON_ACT):
    """Build the per-core Bass program. rows % group == 0, group % 512 == 0."""
    import concourse.bass as bass
    import concourse.tile as tile
    from concourse import bacc, mybir

    F16 = mybir.dt.float16
    F32 = mybir.dt.float32
    AF = mybir.ActivationFunctionType

    nc = bacc.Bacc(None, target_bir_lowering=False)
    xt = nc.declare_dram_parameter("xt", [960, rows], F16, isOutput=False)
    w0 = nc.declare_dram_parameter("w0", [256, 384], F16, isOutput=False)
    w1 = nc.declare_dram_parameter("w1", [128, 64], F16, isOutput=False)
    w2 = nc.declare_dram_parameter("w2", [128, 128], F16, isOutput=False)
    out = nc.declare_dram_parameter("out", [rows, 768], F16, isOutput=True)

    ngroups = rows // group
    npairs = group // 256   # a "pair" = 2 subtiles of 128 rows

    with tile.TileContext(nc) as tc:
        with (
            tc.tile_pool(name="wpool", bufs=1) as wpool,
            tc.tile_pool(name="xpool", bufs=2) as xpool,
            tc.tile_pool(name="gpool", bufs=4) as gpool,
            tc.tile_pool(name="opool", bufs=3) as opool,
            tc.tile_pool(name="ps0", bufs=2, space="PSUM") as ps0,
            tc.tile_pool(name="ps12", bufs=2, space="PSUM") as ps12,
        ):
            w0s = wpool.tile([128, 768], F16, tag="w0")
            w1s = wpool.tile([128, 64], F16, tag="w1")
            w2s = wpool.tile([128, 128], F16, tag="w2")
            nc.sync.dma_start(
                out=w0s[:].rearrange("p (a n) -> p a n", a=2),
                in_=w0.rearrange("(a p) n -> p a n", p=128),
            )
            nc.sync.dma_start(out=w1s[:], in_=w1[:])
            nc.sync.dma_start(out=w2s[:], in_=w2[:])

            for g in range(ngroups):
                r0 = g * group
                xb = []
                for b in range(8):
                    p = 64 if b == 7 else 128
                    t = xpool.tile([p, group], F16, tag=f"xb{b}")
                    nc.sync.dma_start(
                        out=t[:], in_=xt[b * 128 : b * 128 + p, r0 : r0 + group]
                    )
                    xb.append(t)
                for q in range(npairs):
                    c0 = q * 256
                    P0 = ps0.tile([128, 1024], F32, tag="p0")
                    P12 = ps12.tile([128, 1024], F32, tag="p12")
                    Gt = gpool.tile([128, 256], F16, tag="g")
                    GtR = gpool.tile([128, 256], F16, tag="gr")
                    if not silu_on_act:
                        SGt = gpool.tile([128, 512], F16, tag="sg")
                    if q % 2 == 0:
                        O = opool.tile([128, 3072], F16, tag="o")
                        Ov = O[:].rearrange("p (u x) -> p u x", x=768)
                    u0 = (q % 2) * 2
                    for t in range(2):
                        j = c0 + t * 128
                        o512 = t * 512
                        nc.tensor.matmul(
                            P0[:, o512 : o512 + 384], xb[0][:, j : j + 128],
                            w0s[:, 0:384], start=True, stop=False)
                        nc.tensor.matmul(
                            P0[:, o512 : o512 + 384], xb[1][:, j : j + 128],
                            w0s[:, 384:768], start=False, stop=True)
                        for c in range(3):
                            nc.tensor.matmul(
                                P12[:, o512 + c * 64 : o512 + c * 64 + 64],
                                xb[2 + c][:, j : j + 128], w1s[:],
                                start=True, stop=True)
                        nc.tensor.matmul(
                            P12[:, o512 + 192 : o512 + 320],
                            xb[5][:, j : j + 128], w2s[:], start=True, stop=True)
                        nc.tensor.matmul(
                            P12[:, o512 + 320 : o512 + 448],
                            xb[6][:, j : j + 128], w2s[:], start=True, stop=True)
                        nc.tensor.matmul(
                            P12[:, o512 + 448 : o512 + 512],
                            xb[7][:, j : j + 128], w2s[0:64, 0:64],
                            start=True, stop=True)
                    p0v = P0[:].rearrange("p (t x) -> p t x", x=512)
                    p12v = P12[:].rearrange("p (t x) -> p t x", x=512)
                    gv = Gt[:].rearrange("p (t x) -> p t x", x=128)
                    if silu_on_act:
                        # psum holds plain y0; host multiplies by SILU_NORM
                        nc.scalar.activation(
                            Ov[:, u0 : u0 + 2, 0:256], p0v[:, :, 0:256], AF.Silu)
                    else:
                        # psum holds z = SILU_NORM*y0 (folded into w0);
                        # out = z * sigmoid(z/SILU_NORM) == SILU_NORM*silu(y0)
                        sgv = SGt[:].rearrange("p (t x) -> p t x", x=256)
                        nc.scalar.activation(
                            sgv[:, :, :], p0v[:, :, 0:256], AF.Sigmoid,
                            scale=1.0 / SILU_NORM)
                        nc.vector.tensor_mul(
                            Ov[:, u0 : u0 + 2, 0:256], p0v[:, :, 0:256],
                            sgv[:, :, :])
                    # gates: sigmoid(g) = 0.5 + 0.5*tanh(g/2)  (tanh shares
                    # the silu ACT table; sigmoid would force a table swap)
                    grv = GtR[:].rearrange("p (t x) -> p t x", x=128)
                    nc.scalar.activation(
                        grv[:, :, :], p0v[:, :, 256:384], AF.Tanh, scale=0.5)
                    nc.vector.tensor_scalar(
                        Gt[:], GtR[:], 0.5, 0.5,
                        op0=mybir.AluOpType.mult, op1=mybir.AluOpType.add)
                    # y1 * g1, c-major columns (dense dst; host reorders cols)
                    dst1 = Ov[:, u0 : u0 + 2, 256:448].rearrange(
                        "p t (c k) -> p t c k", k=64)
                    src1 = p12v[:, :, 0:192].rearrange("p t (c k) -> p t c k", k=64)
                    g1 = gv[:, :, 0:64].unsqueeze(2).broadcast_to((128, 2, 3, 64))
                    nc.vector.tensor_mul(dst1, src1, g1)
                    dst2 = Ov[:, u0 : u0 + 2, 448:768].rearrange(
                        "p t (c k) -> p t c k", k=64)
                    src2 = p12v[:, :, 192:512].rearrange("p t (c k) -> p t c k", k=64)
                    g2 = gv[:, :, 64:128].unsqueeze(2).broadcast_to((128, 2, 5, 64))
                    nc.vector.tensor_mul(dst2, src2, g2)
                    if q % 2 == 1:
                        s0 = r0 + (q - 1) * 256
                        dst = out[s0 : s0 + 512, :].rearrange(
                            "(u p) c -> p u c", p=128)
                        nc.sync.dma_start(out=dst, in_=Ov[:, :, :])
    nc.finalize()
    return nc


_NC = None


def _get_nc():
    global _NC
    if _NC is None:
        _NC = build_nc(R, GROUP)
    return _NC


def _prep_weights(W0, W1, W2, silu_on_act=SILU_ON_ACT):
    w0p = (np.asarray(W0, np.float32) * INV0).astype(np.float32)
    if not silu_on_act:
        w0p[:, :256] *= SILU_NORM  # z = SILU_NORM*y0 for the silu trick
    w0p = w0p.astype(np.float16)
    w1p = (np.asarray(W1, np.float32) * (INV1 * SIGMOID_NORM)).astype(np.float16)
    w2f = (np.asarray(W2, np.float32) * (INV2 * SIGMOID_NORM)).astype(np.float16)
    w2b = np.zeros((128, 128), np.float16)
    w2b[:64, :64] = w2f
    w2b[64:, 64:] = w2f
    return w0p, w1p, w2b


def _prep_x(x):
    """[N, 960] f32 -> [NCORES, 960, R] f16, channel-permuted + transposed."""
    import jax
    import jax.numpy as jnp

    cpu = jax.devices("cpu")[0]

    def f(xa):
        xp = jnp.pad(xa, ((0, NCORES * R - N), (0, 0)))
        xs = xp.reshape(NCORES, R, 960)
        xg = jnp.take(xs, jnp.asarray(_PERM), axis=2)
        return jnp.transpose(xg, (0, 2, 1)).astype(jnp.float16)

    with jax.default_device(cpu):
        return np.asarray(jax.jit(f)(x))


def _ensure_ntff_hook():
    """Provide antenv.axon_hooks (absent on some agent images) so
    run_bass_kernel_spmd(trace=True) can profile via libaxon_pjrt.so."""
    import sys
    import types
    import ctypes
    import contextlib

    try:
        import antenv.axon_hooks  # noqa: F401
        return
    except ImportError:
        pass
    import antenv

    mod = types.ModuleType("antenv.axon_hooks")
    holder = {"hook": None}
    mod.set_axon_ntff_profile_hook = lambda h: holder.__setitem__("hook", h)
    mod.get_axon_ntff_profile_hook = lambda: holder["hook"]
    sys.modules["antenv.axon_hooks"] = mod
    antenv.axon_hooks = mod

    try:
        lib = ctypes.CDLL("/opt/axon/libaxon_pjrt.so")
        if not hasattr(lib, "axon_start_nrt_profile"):
            return
        lib.axon_start_nrt_profile.argtypes = [
            ctypes.POINTER(ctypes.c_int64), ctypes.c_size_t]
        lib.axon_start_nrt_profile.restype = ctypes.c_int64
        lib.axon_stop_nrt_profile.argtypes = [ctypes.c_char_p]
        lib.axon_stop_nrt_profile.restype = ctypes.c_int64

        @contextlib.contextmanager
        def _hook(output_dir, device_ids):
            import jax
            jax.devices()
            if device_ids:
                ids = (ctypes.c_int64 * len(device_ids))(*device_ids)
                rc = lib.axon_start_nrt_profile(ids, len(device_ids))
            else:
                rc = lib.axon_start_nrt_profile(None, 0)
            if rc != 0:
                raise RuntimeError(f"axon_start_nrt_profile rc={rc}")
            try:
                yield
            finally:
                n = lib.axon_stop_nrt_profile(str(output_dir).encode())
                print(f"ntff profile: {n} file(s) -> {output_dir}")

        mod.set_axon_ntff_profile_hook(_hook)
    except Exception:
        pass


def kernel(x, W0, W1, W2):
    global LAST_RESULT
    from concourse.bass_utils import run_bass_kernel_spmd

    if os.environ.get("BASS_TRACE"):
        _ensure_ntff_hook()

    x = np.asarray(x, np.float32)
    xts = _prep_x(x)
    w0p, w1p, w2b = _prep_weights(W0, W1, W2)
    in_maps = [
        {"xt": xts[k], "w0": w0p, "w1": w1p, "w2": w2b} for k in range(NCORES)
    ]
    res = run_bass_kernel_spmd(
        _get_nc(), in_maps, list(range(NCORES)),
        trace=bool(os.environ.get("BASS_TRACE")),
    )
    LAST_RESULT = res
    o = np.stack([res.results[k]["out"] for k in range(NCORES)], axis=0)
    return _post(o)


def _post(o):
    """[NCORES, R, 768] f16 device output -> [N, 768] f32 final."""
    import jax
    import jax.numpy as jnp

    cpu = jax.devices("cpu")[0]

    def f(oa):
        oa = oa.reshape(NCORES * R, 768)[:N]
        oa = jnp.take(oa, jnp.asarray(_COL_INV), axis=1).astype(jnp.float32)
        if SILU_ON_ACT:
            oa = jnp.concatenate([oa[:, :256] * SILU_NORM, oa[:, 256:]], axis=1)
        return oa

    with jax.default_device(cpu):
        return np.asarray(jax.jit(f)(o))
